# revision 9
# baseline (speedup 1.0000x reference)
"""Binary conv1d + maxpool + per-channel threshold, Trainium2 Bass kernel.

Problem (hardcoded shapes):
  I:  [64, 64, 16384] f32   -> pad L by (3,3) with -1.0, sign()
  W:  [128, 64, 7]    f32   -> sign()
  conv1d (VALID over padded) -> [64, 128, 16384]
  maxpool1d(k=7, s=2)        -> [64, 128, 8189]
  per-channel threshold      -> +-sign outputs

Sharding: data-parallel over batch, 8 batches per core on 8 cores.

Device algorithm per core (8 batches, processed as 4 pairs):
  - ACT computes sign(x) for a 2-batch pair stacked on 128 SBUF partitions
    (batch 2p on partitions 0:64, batch 2p+1 on 64:128) in one pass.
  - Conv as 7 accumulating matmuls (one per tap) with K=64 contract.  The
    two batches run CONCURRENTLY on the two halves of the PE array via
    row-tiling (rhs base_partition 0 / 64), each accumulating into its own
    PSUM banks.
  - DVE evacuates PSUM with a fused pool first stage:
        T[l] = max(c[2l], c[2l+1], c[2l+2])        (2 tensor_tensor ops)
    then maxpool7s2 completes as
        out[l] = max(T[l], T[l+1], T[l+2])         (2 tensor_tensor ops)
  - ACT applies threshold: out = Sign(pooled - t) (per-channel bias), times
    per-channel sign if needed.  Output stored bf16 (+-1 exact), host casts.
"""

import numpy as np

B, Cin, L = 64, 64, 16384
Cout, K = 128, 7
PAD = 3
LPAD = L + 2 * PAD          # 16390
Lp = (L - 7) // 2 + 1       # 8189
NT = Lp + 2                 # 8191 T values
NCORES = 8
BPC = B // NCORES           # 8 batches per core
PAIRS = BPC // 2            # 4

GROUP = 1024                # conv cols per PSUM tile (2 banks, f32)
GSTRIDE = GROUP - 2         # groups overlap by 2 conv cols
NGROUPS = 16                # full groups; cover T[0:8176)
TAIL_S = 2 * (NGROUPS * (GROUP // 2 - 1))   # 16352
TAIL_W = L - TAIL_S         # 32
SIGN_CHUNK = 2048

# how many of the 8 batches run the bf16 pool-tail stages on gpsimd
# (the rest run on vector); balances DVE vs GpSimd
GPSIMD_TAIL_BATCHES = 0

_CACHE = {}


def _build(fast: bool, unit: bool):
    import concourse.mybir as mybir
    from concourse import bacc
    from concourse.tile import TileContext

    f32 = mybir.dt.float32
    bf16 = mybir.dt.bfloat16
    AF = mybir.ActivationFunctionType
    OP = mybir.AluOpType

    nc = bacc.Bacc()
    I_in = nc.declare_dram_parameter("I", [BPC, Cin, L], f32, isOutput=False)
    W_in = nc.declare_dram_parameter("W", [Cout, Cin, K], f32, isOutput=False)
    thr_in = nc.declare_dram_parameter("thr", [Cout, 8], f32, isOutput=False)
    O_out = nc.declare_dram_parameter("O", [BPC, Cout, Lp], bf16, isOutput=True)

    with TileContext(nc) as tc:
        sbufs = 2 if fast else 1
        with (
            tc.tile_pool(name="wpool", bufs=1) as wpool,
            tc.tile_pool(name="spool", bufs=sbufs) as spool,
            tc.tile_pool(name="fpool", bufs=3) as fpool,
            tc.tile_pool(name="tpool", bufs=3 if fast else 2) as tpool,
            tc.tile_pool(name="vpool", bufs=2 if fast else 1) as vpool,
            tc.tile_pool(name="gpool", bufs=1) as gpool,
            tc.tile_pool(name="sepool", bufs=4) as sepool,
            tc.tile_pool(name="pspool", bufs=8, space="PSUM") as pspool,
        ):
            # ---- weight prep: sign(W) as bf16, layout [ci, k*128+co], both halves
            wf = wpool.tile([128, K * Cout], f32, tag="wf")
            w_src = W_in[:].rearrange("co ci k -> ci k co")
            wf_v = wf[:].rearrange("p (k co) -> p k co", co=Cout)
            nc.sync.dma_start(out=wf_v[0:64, :, :], in_=w_src)
            nc.sync.dma_start(out=wf_v[64:128, :, :], in_=w_src)
            wb = wpool.tile([128, K * Cout], bf16, tag="wb")
            nc.scalar.activation(out=wb[0:64, :], in_=wf[0:64, :], func=AF.Sign)
            nc.scalar.activation(out=wb[64:128, :], in_=wf[64:128, :],
                                 func=AF.Sign)

            # ---- thresholds [128, 8] f32
            thr = wpool.tile([128, 8], f32, tag="thr")
            nc.sync.dma_start(out=thr[:, :], in_=thr_in[:])

            groups = [(g * GSTRIDE, GROUP, g * (GROUP // 2 - 1), GROUP // 2 - 1)
                      for g in range(NGROUPS)]
            groups.append((TAIL_S, TAIL_W, NGROUPS * (GROUP // 2 - 1),
                           TAIL_W // 2 - 1))

            batch_idx = 0
            for p in range(PAIRS):
                # ---- sign of both batches of the pair, stacked on partitions
                S = spool.tile([128, LPAD], bf16, tag="S")
                for c0 in range(0, L, SIGN_CHUNK):
                    F = fpool.tile([128, SIGN_CHUNK], f32, tag="F")
                    nc.sync.dma_start(
                        out=F[:, :],
                        in_=I_in[2 * p:2 * p + 2, :, c0:c0 + SIGN_CHUNK]
                        .rearrange("b ci l -> (b ci) l"))
                    nc.scalar.activation(
                        out=S[:, PAD + c0:PAD + c0 + SIGN_CHUNK], in_=F[:, :],
                        func=AF.Sign)
                nc.vector.memset(S[:, 0:PAD], -1.0)
                nc.vector.memset(S[:, PAD + L:LPAD], -1.0)

                # ---- conv + fused pool stage 1 into T buffers
                # Matmuls write even / odd conv columns to separate PSUM
                # tiles (strided rhs).  ScalarE evacuates evens to SBUF;
                # DVE then does max(evens, psum_odds) -> max(., evens+1),
                # respecting the one-PSUM-operand-per-op constraint.
                Sv = S[:].rearrange("p (n two) -> p n two", two=2)

                def s_strided(half, col0, n):
                    return Sv[64 * half:64 * (half + 1),
                              col0 // 2:col0 // 2 + n, col0 % 2]

                Tlo = tpool.tile([128, NT], bf16, tag="T")
                Thi = tpool.tile([128, NT], bf16, tag="T")
                for (s, w, t0, tn) in groups:
                    h = w // 2
                    pse = [pspool.tile([128, h], f32, tag="ps",
                                       name=f"pse{i}_{p}_{s}")
                           for i in range(2)]
                    pso = [pspool.tile([128, h], f32, tag="ps",
                                       name=f"pso{i}_{p}_{s}")
                           for i in range(2)]
                    for tap in range(K):
                        st = (tap == 0)
                        sp = (tap == K - 1)
                        for half in range(2):
                            lw = wb[64 * half:64 * (half + 1),
                                    tap * Cout:(tap + 1) * Cout]
                            nc.tensor.matmul(
                                pse[half][:, 0:h], lw,
                                s_strided(half, s + tap, h),
                                start=st, stop=sp)
                            nc.tensor.matmul(
                                pso[half][:, 0:h], lw,
                                s_strided(half, s + tap + 1, h),
                                start=st, stop=sp)
                    for (half, Tb) in ((0, Tlo), (1, Thi)):
                        SE = sepool.tile([128, GROUP // 2], bf16, tag="SE")
                        nc.scalar.activation(out=SE[:, 0:h], in_=pse[half][:, 0:h],
                                             func=AF.Copy)
                        nc.vector.tensor_tensor(
                            out=Tb[:, t0:t0 + tn], in0=SE[:, 0:tn],
                            in1=pso[half][:, 0:tn], op=OP.max)
                        nc.vector.tensor_tensor(
                            out=Tb[:, t0:t0 + tn], in0=Tb[:, t0:t0 + tn],
                            in1=SE[:, 1:tn + 1], op=OP.max)

                # ---- pool tail + threshold + store, per batch
                for (b, Tb) in ((2 * p, Tlo), (2 * p + 1, Thi)):
                    eng = (nc.gpsimd if batch_idx < GPSIMD_TAIL_BATCHES
                           else nc.vector)
                    batch_idx += 1
                    V = vpool.tile([128, NT - 1], bf16, tag="V")
                    eng.tensor_tensor(out=V[:, 0:NT - 1], in0=Tb[:, 0:NT - 1],
                                      in1=Tb[:, 1:NT], op=OP.max)
                    eng.tensor_tensor(out=V[:, 0:Lp], in0=V[:, 0:Lp],
                                      in1=Tb[:, 2:NT], op=OP.max)
                    if fast:
                        # out = ps * Sign(pooled - tp);  thr col0 = -tp
                        nc.scalar.activation(out=V[:, 0:Lp], in_=V[:, 0:Lp],
                                             func=AF.Sign, bias=thr[:, 0:1])
                        if not unit:
                            nc.vector.tensor_scalar(
                                out=V[:, 0:Lp], in0=V[:, 0:Lp],
                                scalar1=thr[:, 4:5], scalar2=None, op0=OP.mult)
                    else:
                        # general: where(p>=0, where(p>tp, ps, -ps),
                        #                      where(p>tm, ms, -ms))
                        Pp = gpool.tile([128, Lp], bf16, tag="Gp")
                        Pn = gpool.tile([128, Lp], bf16, tag="Gn")
                        G0 = gpool.tile([128, Lp], bf16, tag="G0")
                        nc.vector.tensor_scalar(
                            out=Pp[:, :], in0=V[:, 0:Lp], scalar1=thr[:, 1:2],
                            scalar2=None, op0=OP.is_gt)
                        nc.vector.tensor_scalar(
                            out=Pp[:, :], in0=Pp[:, :], scalar1=thr[:, 3:4],
                            scalar2=thr[:, 4:5], op0=OP.mult, op1=OP.subtract)
                        nc.vector.tensor_scalar(
                            out=Pn[:, :], in0=V[:, 0:Lp], scalar1=thr[:, 2:3],
                            scalar2=None, op0=OP.is_gt)
                        nc.vector.tensor_scalar(
                            out=Pn[:, :], in0=Pn[:, :], scalar1=thr[:, 5:6],
                            scalar2=thr[:, 6:7], op0=OP.mult, op1=OP.subtract)
                        nc.vector.tensor_scalar(
                            out=G0[:, :], in0=V[:, 0:Lp], scalar1=0.0,
                            scalar2=None, op0=OP.is_ge)
                        nc.vector.tensor_tensor(
                            out=Pp[:, :], in0=Pp[:, :], in1=Pn[:, :],
                            op=OP.subtract)
                        nc.vector.tensor_tensor(
                            out=Pp[:, :], in0=G0[:, :], in1=Pp[:, :],
                            op=OP.mult)
                        nc.vector.tensor_tensor(
                            out=V[:, 0:Lp], in0=Pp[:, :], in1=Pn[:, :],
                            op=OP.add)
                    nc.sync.dma_start(out=O_out[b], in_=V[:, 0:Lp])

    nc.compile()
    return nc


def _get_nc(fast, unit):
    key = (fast, unit)
    if key not in _CACHE:
        _CACHE[key] = _build(fast, unit)
    return _CACHE[key]


def kernel(I, W, threshold_plus, threshold_minus, threshold_plus_sign,
           threshold_minus_sign):
    from concourse.bass_utils import run_bass_kernel_spmd

    tp = np.asarray(threshold_plus, dtype=np.float32)
    tm = np.asarray(threshold_minus, dtype=np.float32)
    ps = np.asarray(threshold_plus_sign, dtype=np.float32)
    ms = np.asarray(threshold_minus_sign, dtype=np.float32)
    I = np.ascontiguousarray(np.asarray(I, dtype=np.float32))
    W = np.ascontiguousarray(np.asarray(W, dtype=np.float32))

    fast = np.array_equal(tp, tm) and np.array_equal(ps, ms)
    unit = fast and bool(np.all(ps == 1.0))

    thr = np.zeros((Cout, 8), dtype=np.float32)
    thr[:, 0] = -tp
    thr[:, 1] = tp
    thr[:, 2] = tm
    thr[:, 3] = 2.0 * ps
    thr[:, 4] = ps
    thr[:, 5] = 2.0 * ms
    thr[:, 6] = ms

    nc = _get_nc(fast, unit)
    in_maps = [
        {"I": I[c * BPC:(c + 1) * BPC], "W": W, "thr": thr}
        for c in range(NCORES)
    ]
    res = run_bass_kernel_spmd(nc, in_maps, list(range(NCORES)))
    out = np.concatenate(
        [np.asarray(r["O"]).astype(np.float32) for r in res.results], axis=0)
    return out
